# revision 15
# baseline (speedup 1.0000x reference)
"""Trainium2 Bass kernel for nn_AttentionBlock (dense transformer block).

Data-parallel over batch: each of the 8 NeuronCores processes one batch
element end-to-end (no collectives). Activations are channel-major
(C on partitions, tokens on free). Large matmuls in float32r (TF32-like,
1 cyc/row at N>=256) with fp32 PSUM accumulation. Partition reductions
(layernorm stats, softmax denominators) via ones-vector matmuls; partition
broadcasts via K=1 ones-row matmuls.
"""
import math
import numpy as np
from contextlib import ExitStack

import concourse.bass as bass
import concourse.bacc as bacc
import concourse.mybir as mybir
import concourse.tile as tile

P = 128
C = 640
CT = C // P          # 5
HW = 1024
NHALF = 2
NH = 8               # heads
DH = 80              # head dim
GROUPS = 32
GSIZE = C // GROUPS  # 20
DCTX = 512
LCTX = 77
LCTXP = 80           # padded context length (f32r needs even moving dim)
FFN = 5120
FFH = 2560
FT = FFH // P        # 20

F32 = mybir.dt.float32
F32R = mybir.dt.float32r
AF = mybir.ActivationFunctionType
ALU = mybir.AluOpType
AX = mybir.AxisListType
SCALE = 1.0 / math.sqrt(DH)

_CACHE = {}


def _pcs(dram_ap):
    return dram_ap.rearrange("(t p) -> p t", p=P)


def _build():
    nc = bacc.Bacc("TRN2", target_bir_lowering=False, debug=False)

    xt_d = nc.dram_tensor("xt", [C, HW], F32, kind="ExternalInput")
    ctxT_d = nc.dram_tensor("ctxT", [DCTX, LCTXP], F32R, kind="ExternalInput")

    def w_in(name, shape):
        return nc.dram_tensor(name, shape, F32R, kind="ExternalInput")

    conv1_wT = w_in("conv1_wT", [C, C])
    sa_in_w = w_in("sa_in_w", [C, 3 * C])
    sa_out_w = w_in("sa_out_w", [C, C])
    ca_q_w = w_in("ca_q_w", [C, C])
    ca_k_w = w_in("ca_k_w", [DCTX, C])
    ca_v_w = w_in("ca_v_w", [DCTX, C])
    ca_out_w = w_in("ca_out_w", [C, C])
    lin1_w = w_in("lin1_w", [C, FFN])
    lin2_w = w_in("lin2_w", [FFH, C])
    co_wT = w_in("co_wT", [C, C])
    G_d = w_in("G", [C, GROUPS])
    G2_d = w_in("G2", [GROUPS, C])
    ones_d = w_in("ones128", [P, 1])
    onesrow_d = w_in("onesrow", [1, P])
    vpinit_d = w_in("vpinit", [P, NH * 97])
    vpinit_ca_d = w_in("vpinit_ca", [LCTXP, NH * 97])

    vecs = {}
    for name in ["gn_s", "gn_b", "conv1_b", "ln1_s", "ln1_b", "sa_out_b",
                 "ln2_s", "ln2_b", "ca_out_b", "ln3_s", "ln3_b", "lin2_b", "co_b"]:
        vecs[name] = nc.dram_tensor(name, [C], F32, kind="ExternalInput")
    lin1_b_d = nc.dram_tensor("lin1_b", [FFN], F32, kind="ExternalInput")

    y_d = nc.dram_tensor("y", [C, HW], F32, kind="ExternalOutput")

    with tile.TileContext(nc) as tc, ExitStack() as top:
        cpool = top.enter_context(tc.tile_pool(name="consts", bufs=1))
        respool = top.enter_context(tc.tile_pool(name="resid", bufs=1))

        nvec = len(vecs)
        vpack = cpool.tile([P, nvec * CT + FFN // P + 2], F32, tag="vpack")
        vt = {}
        for i, (name, d) in enumerate(vecs.items()):
            sl = vpack[:, i * CT:(i + 1) * CT]
            nc.sync.dma_start(sl, _pcs(d.ap()))
            vt[name] = sl
        lin1_b_sb = vpack[:, nvec * CT:nvec * CT + FFN // P]
        nc.sync.dma_start(lin1_b_sb, _pcs(lin1_b_d.ap()))
        epsln = vpack[:, nvec * CT + FFN // P:nvec * CT + FFN // P + 1]
        nc.gpsimd.memset(epsln, 1e-5)
        epsgn = vpack[:, nvec * CT + FFN // P + 1:nvec * CT + FFN // P + 2]
        nc.gpsimd.memset(epsgn, 1e-6)
        ones_sb = cpool.tile([P, 1], F32R, tag="ones")
        nc.sync.dma_start(ones_sb[:], ones_d.ap())
        onesrow = cpool.tile([1, P], F32R, tag="onesrow")
        nc.sync.dma_start(onesrow[:], onesrow_d.ap())
        G_sb = cpool.tile([P, CT, GROUPS], F32R, tag="G")
        nc.sync.dma_start(G_sb[:], G_d.ap().rearrange("(t p) g -> p t g", p=P))
        G2_sb = cpool.tile([GROUPS, C], F32R, tag="G2")
        nc.sync.dma_start(G2_sb[:], G2_d.ap())

        # ---------------- helpers ----------------
        def layer_norm(phase_ctx, src, s_vec, b_vec, tag, eps_ap):
            """src: CT [P,HW] f32r tiles -> CT f32r tiles (phase-level pool)."""
            tpool = phase_ctx.enter_context(tc.tile_pool(name=f"t_{tag}", bufs=1))
            out = [tpool.tile([P, HW], F32R, tag=f"t{k}", name=f"t_{tag}{k}")
                   for k in range(CT)]
            with ExitStack() as ctx:
                pool = ctx.enter_context(tc.tile_pool(name=f"ln_{tag}", bufs=1))
                ps = ctx.enter_context(tc.tile_pool(name=f"lnps_{tag}", bufs=1, space="PSUM"))
                bcps = ctx.enter_context(tc.tile_pool(name=f"lnbc_{tag}", bufs=2, space="PSUM"))
                sq = []
                for k in range(CT):
                    sqk = pool.tile([P, HW], F32R, tag=f"sq{k}", name=f"sq{k}")
                    nc.vector.tensor_mul(sqk[:], src[k][:], src[k][:])
                    sq.append(sqk)
                sx_ps = ps.tile([1, HW], F32, tag="sx")
                sxx_ps = ps.tile([1, HW], F32, tag="sxx")
                for n in range(NHALF):
                    nsl = slice(n * 512, (n + 1) * 512)
                    for k in range(CT):
                        nc.tensor.matmul(sx_ps[:, nsl], lhsT=ones_sb[:], rhs=src[k][:, nsl],
                                         start=(k == 0), stop=(k == CT - 1))
                    for k in range(CT):
                        nc.tensor.matmul(sxx_ps[:, nsl], lhsT=ones_sb[:], rhs=sq[k][:, nsl],
                                         start=(k == 0), stop=(k == CT - 1))
                # row stats: mu, A = 1/sqrt(var+eps)  (f32r rows feed bcast matmul)
                mu_row = pool.tile([1, HW], F32R, tag="murow")
                nc.vector.tensor_scalar_mul(mu_row[:], sx_ps[:], 1.0 / C)
                m2_row = pool.tile([1, HW], F32, tag="m2row")
                nc.vector.tensor_scalar_mul(m2_row[:], sxx_ps[:], 1.0 / C)
                mu2_row = pool.tile([1, HW], F32, tag="mu2row")
                nc.vector.tensor_mul(mu2_row[:], mu_row[:], mu_row[:])
                var_row = pool.tile([1, HW], F32, tag="varrow")
                nc.vector.tensor_sub(var_row[:], m2_row[:], mu2_row[:])
                sd_row = pool.tile([1, HW], F32, tag="sdrow")
                nc.scalar.activation(sd_row[:], var_row[:], AF.Sqrt, bias=eps_ap[0:1])
                A_row = pool.tile([1, HW], F32R, tag="Arow")
                with nc.allow_low_precision(reason="f32r rounding of 1/std for bcast matmul"):
                    nc.vector.reciprocal(A_row[:], sd_row[:])
                # broadcast mu, A to all partitions (K=1 matmul)
                mu_bc = bcps.tile([P, HW], F32, tag="bc", name="mu_bc")
                A_bc = bcps.tile([P, HW], F32, tag="bc", name="A_bc")
                for n in range(NHALF):
                    nsl = slice(n * 512, (n + 1) * 512)
                    nc.tensor.matmul(mu_bc[:, nsl], lhsT=onesrow[:], rhs=mu_row[:, nsl],
                                     start=True, stop=True)
                    nc.tensor.matmul(A_bc[:, nsl], lhsT=onesrow[:], rhs=A_row[:, nsl],
                                     start=True, stop=True)
                for k in range(CT):
                    xm = pool.tile([P, HW], F32, tag="xm", name="xm")
                    nc.vector.tensor_sub(xm[:], src[k][:], mu_bc[:])
                    xn = pool.tile([P, HW], F32, tag="xn", name="xn")
                    nc.vector.tensor_mul(xn[:], xm[:], A_bc[:])
                    nc.vector.tensor_scalar(out[k][:], xn[:], s_vec[:, k:k + 1],
                                            b_vec[:, k:k + 1], ALU.mult, ALU.add)
            return out

        def linear_cm(ctx, w_dram, src, cout, tag, consumer, wchunk=None):
            kt = len(src)
            mt_all = cout // P
            wchunk = wchunk or mt_all
            wpool = ctx.enter_context(tc.tile_pool(name=f"w_{tag}", bufs=2))
            ps = ctx.enter_context(tc.tile_pool(name=f"ps_{tag}", bufs=4, space="PSUM"))
            w_ap = w_dram.ap().rearrange("(t p) n -> p t n", p=P)
            for mc in range(0, mt_all, wchunk):
                mhi = min(mc + wchunk, mt_all)
                wt = wpool.tile([P, kt, (mhi - mc) * P], F32R, tag="w", name=f"w_{tag}")
                nc.sync.dma_start(wt[:], w_ap[:, :, mc * P:mhi * P])
                for m in range(mc, mhi):
                    for n in range(NHALF):
                        pst = ps.tile([P, 512], F32, tag="ps", name=f"ps_{tag}")
                        for k in range(kt):
                            nc.tensor.matmul(
                                pst[:], lhsT=wt[:, k, (m - mc) * P:(m - mc + 1) * P],
                                rhs=src[k][:, n * 512:(n + 1) * 512],
                                start=(k == 0), stop=(k == kt - 1))
                        consumer(m, n, pst)

        # ================= Phase 1: GroupNorm + conv1 =================
        x1 = [respool.tile([P, HW], F32R, tag=f"ra{k}", name=f"x1_{k}") for k in range(CT)]
        with ExitStack() as ctx:
            xopool = ctx.enter_context(tc.tile_pool(name="xop", bufs=1))
            x_orig = [xopool.tile([P, HW], F32, tag=f"xo{k}", name=f"xo{k}") for k in range(CT)]
            for k in range(CT):
                nc.sync.dma_start(x_orig[k][:], xt_d.ap()[k * P:(k + 1) * P, :])
            pool = ctx.enter_context(tc.tile_pool(name="gn", bufs=2))
            t0pool = ctx.enter_context(tc.tile_pool(name="t0p", bufs=1))
            with ExitStack() as gctx:
                gps_pool = gctx.enter_context(tc.tile_pool(name="gnps", bufs=1, space="PSUM"))
                scs = pool.tile([P, CT, 2], F32, tag="scs")
                for k in range(CT):
                    nc.vector.reduce_sum(scs[:, k, 0:1], x_orig[k][:], AX.X)
                    sqk = pool.tile([P, HW], F32, tag="gnsq", name="gnsq")
                    nc.vector.tensor_mul(sqk[:], x_orig[k][:], x_orig[k][:])
                    nc.vector.reduce_sum(scs[:, k, 1:2], sqk[:], AX.X)
                scs_r = pool.tile([P, CT, 2], F32R, tag="scsr")
                nc.vector.tensor_scalar_mul(scs_r[:], scs[:], 1.0)
                gps = gps_pool.tile([GROUPS, 2], F32, tag="g")
                for k in range(CT):
                    nc.tensor.matmul(gps[:], lhsT=G_sb[:, k], rhs=scs_r[:, k],
                                     start=(k == 0), stop=(k == CT - 1))
                NG = float(GSIZE * HW)
                gmu = pool.tile([GROUPS, 1], F32, tag="gmu")
                nc.vector.tensor_scalar_mul(gmu[:], gps[:, 0:1], 1.0 / NG)
                gm2 = pool.tile([GROUPS, 1], F32, tag="gm2")
                nc.vector.tensor_scalar_mul(gm2[:], gps[:, 1:2], 1.0 / NG)
                gmu2 = pool.tile([GROUPS, 1], F32, tag="gmu2")
                nc.vector.tensor_mul(gmu2[:], gmu[:], gmu[:])
                gvar = pool.tile([GROUPS, 1], F32, tag="gvar")
                nc.vector.tensor_sub(gvar[:], gm2[:], gmu2[:])
                gsd = pool.tile([GROUPS, 1], F32, tag="gsd")
                nc.scalar.activation(gsd[:], gvar[:], AF.Sqrt, bias=epsgn[:GROUPS])
                gA_f = pool.tile([GROUPS, 1], F32, tag="gAf")
                nc.vector.reciprocal(gA_f[:], gsd[:])
                gAB = pool.tile([GROUPS, 2], F32R, tag="gAB")
                nc.vector.tensor_scalar_mul(gAB[:, 0:1], gA_f[:], 1.0)
                gB_f = pool.tile([GROUPS, 1], F32, tag="gBf")
                nc.vector.tensor_mul(gB_f[:], gmu[:], gA_f[:])
                nc.vector.tensor_scalar_mul(gAB[:, 1:2], gB_f[:], -1.0)
                t0 = []
                for k in range(CT):
                    cps = gps_pool.tile([P, 2], F32, tag="cps")
                    nc.tensor.matmul(cps[:], lhsT=G2_sb[:, k * P:(k + 1) * P], rhs=gAB[:],
                                     start=True, stop=True)
                    cA = pool.tile([P, 1], F32, tag="cA", name="cA")
                    nc.vector.tensor_mul(cA[:], cps[:, 0:1], vt["gn_s"][:, k:k + 1])
                    cB = pool.tile([P, 1], F32, tag="cB", name="cB")
                    nc.vector.tensor_mul(cB[:], cps[:, 1:2], vt["gn_s"][:, k:k + 1])
                    nc.vector.tensor_add(cB[:], cB[:], vt["gn_b"][:, k:k + 1])
                    o = t0pool.tile([P, HW], F32R, tag=f"t0_{k}", name=f"t0_{k}")
                    nc.vector.tensor_scalar(o[:], x_orig[k][:], cA[:], cB[:],
                                            ALU.mult, ALU.add)
                    t0.append(o)

            def conv1_consumer(m, n, pst):
                nsl = slice(n * 512, (n + 1) * 512)
                nc.vector.tensor_scalar_add(x1[m][:, nsl], pst[:], vt["conv1_b"][:, m:m + 1])
            linear_cm(ctx, conv1_wT, t0, C, "conv1", conv1_consumer)

        # ================= Phase 2: LN1 + self-attention =================
        x2 = [respool.tile([P, HW], F32R, tag=f"rb{k}", name=f"x2_{k}") for k in range(CT)]
        with ExitStack() as ctx:
            t1 = layer_norm(ctx, x1, vt["ln1_s"], vt["ln1_b"], "ln1", epsln)

            wqkp = ctx.enter_context(tc.tile_pool(name="wqkp", bufs=1))
            wv = ctx.enter_context(tc.tile_pool(name="savw", bufs=1))
            vpool = ctx.enter_context(tc.tile_pool(name="vp", bufs=1))
            qk_sb = ctx.enter_context(tc.tile_pool(name="qksb", bufs=2))
            expp = ctx.enter_context(tc.tile_pool(name="expp", bufs=3))
            ohp = ctx.enter_context(tc.tile_pool(name="ohp", bufs=1))
            recp = ctx.enter_context(tc.tile_pool(name="recp", bufs=2))
            rbp = ctx.enter_context(tc.tile_pool(name="rbp", bufs=1))

            sa_in_ap = sa_in_w.ap().rearrange("(t p) n -> p t n", p=P)
            oh = ohp.tile([DH, NH, HW], F32R, tag="oh")
            qt, kt_ = {}, {}

            # full Q/K weights, one DMA each (contiguous 2.5KB row chunks)
            wq_sb = wqkp.tile([P, CT, C], F32R, tag="wq")
            nc.sync.dma_start(wq_sb[:], sa_in_ap[:, :, 0:C])
            wk_sb = wqkp.tile([P, CT, C], F32R, tag="wk")
            nc.sync.dma_start(wk_sb[:], sa_in_ap[:, :, C:2 * C])

            with ExitStack() as actx:
                ps_sqk = actx.enter_context(tc.tile_pool(name="ps_sqk", bufs=3, space="PSUM"))
                ps_o = actx.enter_context(tc.tile_pool(name="ps_o", bufs=1, space="PSUM"))

                wv_sb = wv.tile([P, CT, C], F32R, tag="wvwo", name="wv_sb")
                nc.sync.dma_start(wv_sb[:], sa_in_ap[:, :, 2 * C:3 * C])
                vp = [vpool.tile([P, NH * 97], F32R, tag=f"vp{mk}", name=f"vp{mk}")
                      for mk in range(NH)]
                for mk in range(NH):
                    nc.sync.dma_start(vp[mk][:], vpinit_d.ap())
                    for nb in range(2):
                        vps = ps_sqk.tile([P, 320], F32, tag="sps", name="vps")
                        for k in range(CT):
                            nc.tensor.matmul(vps[:], lhsT=t1[k][:, mk * P:(mk + 1) * P],
                                             rhs=wv_sb[:, k, nb * 320:(nb + 1) * 320],
                                             start=(k == 0), stop=(k == CT - 1))
                        for h in range(nb * 4, nb * 4 + 4):
                            nc.vector.tensor_scalar_mul(
                                vp[mk][:, h * 97:h * 97 + DH],
                                vps[:, (h - nb * 4) * DH:(h - nb * 4 + 1) * DH], 1.0)

                def project_qk(h):
                    qp = ps_sqk.tile([P, HW], F32, tag="sps", name="qps")
                    kp = ps_sqk.tile([P, HW], F32, tag="sps", name="kps")
                    for n in range(NHALF):
                        nsl = slice(n * 512, (n + 1) * 512)
                        for k in range(CT):
                            nc.tensor.matmul(qp[:DH, nsl],
                                             lhsT=wq_sb[:, k, h * DH:(h + 1) * DH],
                                             rhs=t1[k][:, nsl], start=(k == 0),
                                             stop=(k == CT - 1))
                        for k in range(CT):
                            nc.tensor.matmul(kp[:DH, nsl],
                                             lhsT=wk_sb[:, k, h * DH:(h + 1) * DH],
                                             rhs=t1[k][:, nsl], start=(k == 0),
                                             stop=(k == CT - 1))
                    q = qk_sb.tile([DH, HW], F32R, tag="qt", name="qtile")
                    nc.vector.tensor_scalar_mul(q[:], qp[:DH], SCALE)
                    kk = qk_sb.tile([DH, HW], F32R, tag="kt", name="ktile")
                    nc.vector.tensor_scalar_mul(kk[:], kp[:DH], 1.0)
                    qt[h], kt_[h] = q, kk

                project_qk(0)
                for h in range(NH):
                    exps = []
                    for mk in range(NH):
                        sps = ps_sqk.tile([P, HW], F32, tag="sps", name="sps")
                        for n in range(NHALF):
                            nsl = slice(n * 512, (n + 1) * 512)
                            nc.tensor.matmul(sps[:, nsl],
                                             lhsT=kt_[h][:, mk * P:(mk + 1) * P],
                                             rhs=qt[h][:, nsl], start=True, stop=True)
                        e = expp.tile([P, HW], F32R, tag="exps", name="exps")
                        nc.scalar.activation(e[:], sps[:], AF.Exp)
                        exps.append(e)
                    if h + 1 < NH:
                        project_qk(h + 1)
                    ops_ = ps_o.tile([97, HW], F32, tag="ops")
                    for mk in range(NH):
                        for n in range(NHALF):
                            nsl = slice(n * 512, (n + 1) * 512)
                            nc.tensor.matmul(ops_[:, nsl],
                                             lhsT=vp[mk][:, h * 97:(h + 1) * 97],
                                             rhs=exps[mk][:, nsl],
                                             start=(mk == 0), stop=(mk == NH - 1))
                    rec = recp.tile([1, HW], F32R, tag="rec", name="rec")
                    with nc.allow_low_precision(reason="f32r rounding of softmax denom"):
                        nc.vector.reciprocal(rec[:], ops_[96:97, :])
                    rbps = ps_sqk.tile([P, HW], F32, tag="sps", name="rbps")
                    for n in range(NHALF):
                        nsl = slice(n * 512, (n + 1) * 512)
                        nc.tensor.matmul(rbps[:DH, nsl], lhsT=onesrow[:, :DH],
                                         rhs=rec[:, nsl], start=True, stop=True)
                    rb = rbp.tile([DH, HW], F32, tag="rb", name="rb")
                    nc.vector.tensor_copy(rb[:], rbps[:DH])
                    nc.vector.tensor_mul(oh[:, h, :], ops_[:DH, :], rb[:])

            wo_sb = wv.tile([DH, NH, C], F32R, tag="wvwo", name="wo_sb")
            nc.sync.dma_start(wo_sb[:], sa_out_w.ap().rearrange("(h d) n -> d h n", d=DH))
            with ExitStack() as octx:
                ps_out = octx.enter_context(tc.tile_pool(name="ps_saout", bufs=4, space="PSUM"))
                for m in range(CT):
                    for n in range(NHALF):
                        nsl = slice(n * 512, (n + 1) * 512)
                        pst = ps_out.tile([P, 512], F32, tag="po", name="po")
                        for h in range(NH):
                            nc.tensor.matmul(pst[:], lhsT=wo_sb[:, h, m * P:(m + 1) * P],
                                             rhs=oh[:, h, nsl],
                                             start=(h == 0), stop=(h == NH - 1))
                        nc.vector.scalar_tensor_tensor(
                            x2[m][:, nsl], pst[:], vt["sa_out_b"][:, m:m + 1],
                            x1[m][:, nsl], ALU.add, ALU.add)

        # ================= Phase 3: LN2 + cross-attention =================
        x3 = [respool.tile([P, HW], F32R, tag=f"ra{k}", name=f"x3_{k}") for k in range(CT)]
        with ExitStack() as ctx:
            t2 = layer_norm(ctx, x2, vt["ln2_s"], vt["ln2_b"], "ln2", epsln)

            capool = ctx.enter_context(tc.tile_pool(name="ca", bufs=1))
            caw = ctx.enter_context(tc.tile_pool(name="caw", bufs=1))
            wqcap = ctx.enter_context(tc.tile_pool(name="wqcap", bufs=1))
            qcap = ctx.enter_context(tc.tile_pool(name="qca", bufs=2))
            expca = ctx.enter_context(tc.tile_pool(name="expca", bufs=3))
            recp = ctx.enter_context(tc.tile_pool(name="carecp", bufs=2))
            rbp = ctx.enter_context(tc.tile_pool(name="carbp", bufs=1))

            ohca = capool.tile([DH, NH, HW], F32R, tag="ohca")
            qtc = {}

            with ExitStack() as actx:
                ps_ca = actx.enter_context(tc.tile_pool(name="ps_ca", bufs=3, space="PSUM"))
                ps_oca = actx.enter_context(tc.tile_pool(name="ps_oca", bufs=1, space="PSUM"))

                ctx_sb = capool.tile([P, 4, LCTXP], F32R, tag="ctx")
                nc.sync.dma_start(ctx_sb[:], ctxT_d.ap().rearrange("(t p) n -> p t n", p=P))
                kca = capool.tile([DH, NH, LCTXP], F32R, tag="kca")
                wk_sb = caw.tile([P, 4, C], F32R, tag="cawbig", name="wk_ca")
                nc.sync.dma_start(wk_sb[:], ca_k_w.ap().rearrange("(t p) n -> p t n", p=P))
                for h in range(NH):
                    kps = ps_ca.tile([DH, LCTXP], F32, tag="caps", name="kps_ca")
                    for k in range(4):
                        nc.tensor.matmul(kps[:], lhsT=wk_sb[:, k, h * DH:(h + 1) * DH],
                                         rhs=ctx_sb[:, k, :], start=(k == 0), stop=(k == 3))
                    nc.vector.tensor_scalar_mul(kca[:, h, :], kps[:], 1.0)
                wvca_sb = caw.tile([P, 4, C], F32R, tag="cawbig", name="wv_ca")
                nc.sync.dma_start(wvca_sb[:], ca_v_w.ap().rearrange("(t p) n -> p t n", p=P))
                vca = capool.tile([LCTXP, NH * 97], F32R, tag="vca")
                nc.sync.dma_start(vca[:], vpinit_ca_d.ap())
                for nb in range(2):
                    vps = ps_ca.tile([LCTXP, 320], F32, tag="caps", name="vps_ca")
                    for k in range(4):
                        nc.tensor.matmul(vps[:], lhsT=ctx_sb[:, k, :],
                                         rhs=wvca_sb[:, k, nb * 320:(nb + 1) * 320],
                                         start=(k == 0), stop=(k == 3))
                    for h in range(nb * 4, nb * 4 + 4):
                        nc.vector.tensor_scalar_mul(
                            vca[:, h * 97:h * 97 + DH],
                            vps[:, (h - nb * 4) * DH:(h - nb * 4 + 1) * DH], 1.0)

                wqca_sb = wqcap.tile([P, CT, C], F32R, tag="wqca")
                nc.sync.dma_start(wqca_sb[:], ca_q_w.ap().rearrange("(t p) n -> p t n", p=P))

                def project_q_ca(h):
                    qp = ps_ca.tile([DH, HW], F32, tag="caps", name="qps_ca")
                    for n in range(NHALF):
                        nsl = slice(n * 512, (n + 1) * 512)
                        for k in range(CT):
                            nc.tensor.matmul(qp[:, nsl],
                                             lhsT=wqca_sb[:, k, h * DH:(h + 1) * DH],
                                             rhs=t2[k][:, nsl],
                                             start=(k == 0), stop=(k == CT - 1))
                    q = qcap.tile([DH, HW], F32R, tag="qtca", name="qtca")
                    nc.vector.tensor_scalar_mul(q[:], qp[:], SCALE)
                    qtc[h] = q

                project_q_ca(0)
                for h in range(NH):
                    sps = ps_ca.tile([LCTXP, HW], F32, tag="caps", name="sps_ca")
                    for n in range(NHALF):
                        nsl = slice(n * 512, (n + 1) * 512)
                        nc.tensor.matmul(sps[:, nsl], lhsT=kca[:, h, :], rhs=qtc[h][:, nsl],
                                         start=True, stop=True)
                    e = expca.tile([LCTXP, HW], F32R, tag="expca", name="expca_t")
                    nc.scalar.activation(e[:], sps[:], AF.Exp)
                    if h + 1 < NH:
                        project_q_ca(h + 1)
                    ops_ = ps_oca.tile([97, HW], F32, tag="opsca")
                    for n in range(NHALF):
                        nsl = slice(n * 512, (n + 1) * 512)
                        nc.tensor.matmul(ops_[:, nsl], lhsT=vca[:, h * 97:(h + 1) * 97],
                                         rhs=e[:, nsl], start=True, stop=True)
                    rec = recp.tile([1, HW], F32R, tag="recca", name="recca")
                    with nc.allow_low_precision(reason="f32r rounding of softmax denom"):
                        nc.vector.reciprocal(rec[:], ops_[96:97, :])
                    rbps = ps_ca.tile([P, HW], F32, tag="caps", name="rbps_ca")
                    for n in range(NHALF):
                        nsl = slice(n * 512, (n + 1) * 512)
                        nc.tensor.matmul(rbps[:DH, nsl], lhsT=onesrow[:, :DH],
                                         rhs=rec[:, nsl], start=True, stop=True)
                    rb = rbp.tile([DH, HW], F32, tag="rbca", name="rbca")
                    nc.vector.tensor_copy(rb[:], rbps[:DH])
                    nc.vector.tensor_mul(ohca[:, h, :], ops_[:DH, :], rb[:])

            woca_sb = caw.tile([DH, NH, C], F32R, tag="cawbig", name="wo_ca")
            nc.sync.dma_start(woca_sb[:], ca_out_w.ap().rearrange("(h d) n -> d h n", d=DH))
            with ExitStack() as octx:
                ps_out = octx.enter_context(tc.tile_pool(name="ps_caout", bufs=4, space="PSUM"))
                for m in range(CT):
                    for n in range(NHALF):
                        nsl = slice(n * 512, (n + 1) * 512)
                        pst = ps_out.tile([P, 512], F32, tag="poca", name="poca")
                        for h in range(NH):
                            nc.tensor.matmul(pst[:], lhsT=woca_sb[:, h, m * P:(m + 1) * P],
                                             rhs=ohca[:, h, nsl],
                                             start=(h == 0), stop=(h == NH - 1))
                        nc.vector.scalar_tensor_tensor(
                            x3[m][:, nsl], pst[:], vt["ca_out_b"][:, m:m + 1],
                            x2[m][:, nsl], ALU.add, ALU.add)

        # ================= Phase 4: LN3 + GeGLU FFN (+ conv out) =================
        x4 = [respool.tile([P, HW], F32R, tag=f"rb{k}", name=f"x4_{k}") for k in range(CT)]
        with ExitStack() as ctx:
            t3 = layer_norm(ctx, x3, vt["ln3_s"], vt["ln3_b"], "ln3", epsln)

            with ExitStack() as fctx:
                gpool = fctx.enter_context(tc.tile_pool(name="geglu", bufs=3))
                apool = fctx.enter_context(tc.tile_pool(name="a_tmp", bufs=3))
                w1pool = fctx.enter_context(tc.tile_pool(name="w1", bufs=3))
                w2pool = fctx.enter_context(tc.tile_pool(name="w2", bufs=1))
                ps_f = fctx.enter_context(tc.tile_pool(name="ps_ffn", bufs=3, space="PSUM"))
                ps_l2 = fctx.enter_context(tc.tile_pool(name="ps_l2", bufs=5, space="PSUM"))

                lin1_ap = lin1_w.ap().rearrange("(t p) n -> p t n", p=P)
                lin2_ap = lin2_w.ap().rearrange("(t p) n -> p t n", p=P)
                w2_sb = w2pool.tile([P, FT, C], F32R, tag="w2t")
                nc.sync.dma_start(w2_sb[:], lin2_ap)

                for n in range(NHALF):
                    nsl = slice(n * 512, (n + 1) * 512)
                    l2ps = [ps_l2.tile([P, 512], F32, tag="l2ps", name=f"l2ps{m}")
                            for m in range(CT)]
                    for c in range(4):  # 640-col weight chunks
                        wa = w1pool.tile([P, CT, C], F32R, tag="w1t", name="w1a")
                        nc.sync.dma_start(wa[:], lin1_ap[:, :, c * C:(c + 1) * C])
                        wg = w1pool.tile([P, CT, C], F32R, tag="w1t", name="w1g")
                        nc.sync.dma_start(wg[:], lin1_ap[:, :, FFH + c * C:FFH + (c + 1) * C])
                        for j in range(CT):  # 5 gate tiles per chunk
                            i = c * CT + j
                            aps = ps_f.tile([P, 512], F32, tag="fps", name="aps")
                            for k in range(CT):
                                nc.tensor.matmul(aps[:], lhsT=wa[:, k, j * P:(j + 1) * P],
                                                 rhs=t3[k][:, nsl],
                                                 start=(k == 0), stop=(k == CT - 1))
                            gps = ps_f.tile([P, 512], F32, tag="fps", name="gps")
                            for k in range(CT):
                                nc.tensor.matmul(gps[:], lhsT=wg[:, k, j * P:(j + 1) * P],
                                                 rhs=t3[k][:, nsl],
                                                 start=(k == 0), stop=(k == CT - 1))
                            a_sb = apool.tile([P, 512], F32, tag="a", name="a_sb")
                            nc.vector.tensor_scalar_add(a_sb[:], aps[:], lin1_b_sb[:, i:i + 1])
                            g_sb = apool.tile([P, 512], F32, tag="gg", name="g_sb")
                            nc.scalar.activation(g_sb[:], gps[:], AF.Gelu,
                                                 bias=lin1_b_sb[:, FT + i:FT + i + 1])
                            gi = gpool.tile([P, 512], F32R, tag="g", name="gi")
                            nc.vector.tensor_mul(gi[:], a_sb[:], g_sb[:])
                            for m in range(CT):
                                nc.tensor.matmul(l2ps[m][:],
                                                 lhsT=w2_sb[:, i, m * P:(m + 1) * P],
                                                 rhs=gi[:],
                                                 start=(i == 0), stop=(i == FT - 1))
                    for m in range(CT):
                        nc.vector.scalar_tensor_tensor(
                            x4[m][:, nsl], l2ps[m][:], vt["lin2_b"][:, m:m + 1],
                            x3[m][:, nsl], ALU.add, ALU.add)

            # ---- conv out + long residual ----
            opool = ctx.enter_context(tc.tile_pool(name="outp", bufs=3))
            xo2pool = ctx.enter_context(tc.tile_pool(name="xo2p", bufs=1))
            xo2 = [xo2pool.tile([P, HW], F32, tag=f"xo2_{k}", name=f"xo2_{k}")
                   for k in range(CT)]
            for k in range(CT):
                nc.sync.dma_start(xo2[k][:], xt_d.ap()[k * P:(k + 1) * P, :])

            def co_consumer(m, n, pst):
                nsl = slice(n * 512, (n + 1) * 512)
                o = opool.tile([P, 512], F32, tag="osb", name="osb")
                nc.vector.scalar_tensor_tensor(o[:], pst[:], vt["co_b"][:, m:m + 1],
                                               xo2[m][:, nsl], ALU.add, ALU.add)
                nc.sync.dma_start(y_d.ap()[m * P:(m + 1) * P, nsl], o[:])
            linear_cm(ctx, co_wT, x4, C, "co", co_consumer)

    nc.compile()
    return nc


def _get_program():
    if "nc" not in _CACHE:
        _CACHE["nc"] = _build()
    return _CACHE["nc"]


def _make_runner(nc, n_cores=8):
    import jax
    import numpy as _np
    from jax.experimental.shard_map import shard_map
    from jax.sharding import Mesh, PartitionSpec, NamedSharding
    from concourse import bass2jax
    import concourse.mybir as _mybir

    bass2jax.install_neuronx_cc_hook()
    partition_name = nc.partition_id_tensor.name if nc.partition_id_tensor else None

    in_names, out_names, out_avals, zero_outs = [], [], [], []
    for alloc in nc.m.functions[0].allocations:
        if not isinstance(alloc, _mybir.MemoryLocationSet):
            continue
        name = alloc.memorylocations[0].name
        if alloc.kind == "ExternalInput":
            if name != partition_name:
                in_names.append(name)
        elif alloc.kind == "ExternalOutput":
            shape = tuple(alloc.tensor_shape)
            dtype = _mybir.dt.np(alloc.dtype)
            out_names.append(name)
            out_avals.append(jax.core.ShapedArray(shape, dtype))
            zero_outs.append(_np.zeros(shape, dtype))
    n_params = len(in_names)
    n_outs = len(out_avals)
    all_in_names = list(in_names) + list(out_names)
    if partition_name is not None:
        all_in_names.append(partition_name)

    def _body(*args):
        operands = list(args)
        if partition_name is not None:
            operands.append(bass2jax.partition_id_tensor())
        outs = bass2jax._bass_exec_p.bind(
            *operands,
            out_avals=tuple(out_avals),
            in_names=tuple(all_in_names),
            out_names=tuple(out_names),
            lowering_input_output_aliases=(),
            sim_require_finite=True,
            sim_require_nnan=True,
            nc=nc,
        )
        return tuple(outs)

    devices = jax.devices()[:n_cores]
    mesh = Mesh(_np.asarray(devices), ("core",))
    in_specs = (PartitionSpec("core"),) * (n_params + n_outs)
    out_specs = (PartitionSpec("core"),) * n_outs
    sharded = jax.jit(
        shard_map(_body, mesh=mesh, in_specs=in_specs, out_specs=out_specs,
                  check_rep=False),
        keep_unused=True)
    shard = NamedSharding(mesh, PartitionSpec("core"))

    def prepare(in_maps):
        per_core = [[_np.asarray(m[name]) for name in in_names] for m in in_maps]
        concat_in = [_np.concatenate([per_core[c][i] for c in range(n_cores)], axis=0)
                     for i in range(n_params)]
        concat_zeros = [_np.zeros((n_cores * z.shape[0], *z.shape[1:]), z.dtype)
                        for z in zero_outs]
        dev = [jax.device_put(a, shard) for a in concat_in + concat_zeros]
        jax.block_until_ready(dev)
        return dev

    def execute(dev_args, block=True):
        out_arrs = sharded(*dev_args)
        if block:
            jax.block_until_ready(out_arrs)
        return out_arrs

    def run(in_maps, want_outputs=True):
        out_arrs = execute(prepare(in_maps))
        if not want_outputs:
            return None
        return [
            {name: _np.asarray(out_arrs[i]).reshape(n_cores, *out_avals[i].shape)[c]
             for i, name in enumerate(out_names)}
            for c in range(n_cores)
        ]

    run.in_names = in_names
    run.prepare = prepare
    run.execute = execute
    return run


def _get_runner():
    if "runner" not in _CACHE:
        _CACHE["runner"] = _make_runner(_get_program())
    return _CACHE["runner"]


def _vpinit(rows, valid=None):
    v = np.zeros((rows, NH * 97), np.float32)
    for h in range(NH):
        v[:valid if valid else rows, h * 97 + 96] = 1.0
    return v


def _make_in_maps(inputs):
    x = np.asarray(inputs["x"], dtype=np.float32)
    context = np.asarray(inputs["context"], dtype=np.float32)
    B = x.shape[0]

    G = np.zeros((C, GROUPS), np.float32)
    for c in range(C):
        G[c, c // GSIZE] = 1.0
    shared = {
        "conv1_wT": np.ascontiguousarray(np.asarray(inputs["conv1_w"], np.float32).T),
        "co_wT": np.ascontiguousarray(np.asarray(inputs["co_w"], np.float32).T),
        "G": G, "G2": np.ascontiguousarray(G.T),
        "ones128": np.ones((P, 1), np.float32),
        "onesrow": np.ones((1, P), np.float32),
        "vpinit": _vpinit(P),
        "vpinit_ca": _vpinit(LCTXP, LCTX),
    }
    for name in ["sa_in_w", "sa_out_w", "ca_q_w", "ca_k_w", "ca_v_w", "ca_out_w",
                 "lin1_w", "lin2_w", "gn_s", "gn_b", "conv1_b", "ln1_s", "ln1_b",
                 "sa_out_b", "ln2_s", "ln2_b", "ca_out_b", "ln3_s", "ln3_b",
                 "lin1_b", "lin2_b", "co_b"]:
        shared[name] = np.ascontiguousarray(np.asarray(inputs[name], np.float32))

    in_maps = []
    for b in range(B):
        m = dict(shared)
        m["xt"] = np.ascontiguousarray(x[b].reshape(C, HW))
        ct = np.zeros((DCTX, LCTXP), np.float32)
        ct[:, :LCTX] = context[b].T
        m["ctxT"] = ct
        in_maps.append(m)
    return in_maps


def kernel(**inputs) -> np.ndarray:
    run = _get_runner()
    in_maps = _make_in_maps(inputs)
    results = run(in_maps)
    out = np.stack([results[b]["y"] for b in range(8)], axis=0)
    return out.reshape(8, C, 32, 32).astype(np.float32)
